# revision 8
# baseline (speedup 1.0000x reference)
"""Trainium2 Bass kernel for nn_MultiHeadAttention_22144851378311.

Computation (per batch element b, one NeuronCore each; B=8 = n_cores):
    s = LN(src); t = LN(tgt)
    k = s@k_w.T ; q = t@q_w.T ; v = relu(t@v_w1.T+v_b1)@v_w2.T+v_b2
    S[h,i,j] = q_h[i]. k_h[j] ; qr[h,i,p] = q_h[i] . rel_k[p]
    attn = softmax((S + qr[gpm]) / 8) with key mask
    out  = (attn @ v_h  concat heads) * sigmoid(s@gate_w.T+gate_b) @ out_w.T + out_b

Host-side prep is layout/dtype only: LN affine params are folded into the
projection weights (exact algebra), gate with gate_w==0 folds sigmoid(gate_b)
into out_w columns (general gate_w path computed on-chip), weights are
pre-transposed into lhsT layout and cast to bf16.
"""

import sys

for _p in ("/opt/trn_rl_repo",):
    if _p not in sys.path:
        sys.path.insert(0, _p)

import numpy as np
import ml_dtypes

import concourse.bass as bass
import concourse.bacc as bacc
import concourse.tile as tile
from concourse import mybir
from concourse.bass_utils import run_bass_kernel_spmd

F32 = mybir.dt.float32
BF16 = mybir.dt.bfloat16
AF = mybir.ActivationFunctionType
ALU = mybir.AluOpType

B, L, D, H, PK = 8, 512, 1024, 16, 5
DK = D // H          # 64
NI = L // 128        # 4 i-tiles
ND = D // 128        # 8 d-blocks
SCALE = 1.0 / 8.0    # 1/sqrt(DK)
EPS = 1e-6

_PROG_CACHE = {}


def _ap_bcast_rows(ap, nrows):
    """Broadcast a 1-row DRAM AP across `nrows` partitions (step-0 partition dim)."""
    return bass.AP(tensor=ap.tensor, offset=ap.offset, ap=[[0, nrows]] + list(ap.ap))


def build_program(honest_gate: bool, use_mask: bool, reps: int = 1):
    nc = bacc.Bacc("TRN2", target_bir_lowering=False, debug=False, num_devices=8)

    din = {}
    def dram_in(name, shape, dt):
        din[name] = nc.dram_tensor(name, list(shape), dt, kind="ExternalInput").ap()
        return din[name]

    xs = dram_in("xs", (L, D), F32)          # src[b]
    xt = dram_in("xt", (L, D), F32)          # tgt[b]
    gpm_f = dram_in("gpm_f", (L, L), F32)    # gpm[b] as f32 (values 0..4)
    wkT = dram_in("wkT", (D, D), BF16)       # (k_w * g).T  == lhsT [in, out]
    wqT = dram_in("wqT", (D, D), BF16)
    wv1T = dram_in("wv1T", (D, D), BF16)
    wv2T = dram_in("wv2T", (D, D), BF16)
    woT = dram_in("woT", (D, D), BF16)       # (out_w * gate0).T on fast path
    relkT2 = dram_in("relkT2", (128, PK), BF16)   # rel_k.T stacked twice
    kb = dram_in("kb", (D,), F32)
    qb = dram_in("qb", (D,), F32)
    v1b = dram_in("v1b", (D,), F32)
    v2b = dram_in("v2b", (D,), F32)
    ob = dram_in("ob", (D,), F32)
    if honest_gate:
        wgT = dram_in("wgT", (D, D), BF16)
        gb = dram_in("gb", (D,), F32)
    if use_mask:
        mbias = dram_in("mbias", (L,), F32)  # 0 where kept, -9e9 where masked

    yT = nc.dram_tensor("yT", [D, L], F32, kind="ExternalOutput").ap()

    with tile.TileContext(nc) as tc:
        for _ in range(reps):
            _emit_body(nc, tc, din, yT, honest_gate, use_mask)

    nc.compile()
    return nc


def _emit_body(nc, tc, din, yT, honest_gate, use_mask):
    from contextlib import ExitStack

    xs, xt, gpm_f = din["xs"], din["xt"], din["gpm_f"]
    relkT2_d = din["relkT2"]

    with ExitStack() as ctx:
        ec = ctx.enter_context
        const = ec(tc.tile_pool(name="const", bufs=1))
        lnx = ec(tc.tile_pool(name="lnx", bufs=3))
        stat = ec(tc.tile_pool(name="stat", bufs=8))
        xh = ec(tc.tile_pool(name="xh", bufs=2))
        big = ec(tc.tile_pool(name="big", bufs=1))
        wp = ec(tc.tile_pool(name="wp", bufs=2))
        bp = ec(tc.tile_pool(name="bp", bufs=4))
        accp = ec(tc.tile_pool(name="accp", bufs=4))
        pp = ec(tc.tile_pool(name="pp", bufs=4))
        ptp = ec(tc.tile_pool(name="ptp", bufs=3))
        sevp = ec(tc.tile_pool(name="sevp", bufs=2))
        tiny = ec(tc.tile_pool(name="tiny", bufs=10))
        ytp = ec(tc.tile_pool(name="ytp", bufs=2))
        ps_a = ec(tc.tile_pool(name="ps_a", bufs=2, space="PSUM"))
        ps_qr = ec(tc.tile_pool(name="ps_qr", bufs=1, space="PSUM"))
        ps_s = ec(tc.tile_pool(name="ps_s", bufs=3, space="PSUM"))
        ps_o = ec(tc.tile_pool(name="ps_o", bufs=2, space="PSUM"))

        eps_t = const.tile([128, 1], F32)
        nc.vector.memset(eps_t, EPS)
        relkT2 = const.tile([128, PK], BF16)
        nc.sync.dma_start(out=relkT2, in_=relkT2_d)

        # persistent activations (transposed layouts: [d partition-block, token])
        xsT = big.tile([128, ND, L], BF16, tag="xsT")
        xtT = big.tile([128, ND, L], BF16, tag="xtT")
        qTh = big.tile([64, H, L], BF16, tag="qTh")
        kTh = big.tile([64, H, L], BF16, tag="kTh")
        v1T = big.tile([128, ND, L], BF16, tag="v1T")
        vT = big.tile([128, ND, L], BF16, tag="vT")
        vnat = big.tile([128, NI, D], BF16, tag="vnat")   # v natural [j, d]
        outT = big.tile([128, ND, L], BF16, tag="outT")   # attention out ^T
        masks = big.tile([128, NI, 4, L], BF16, tag="masks")
        qrsb = big.tile([128, NI, H * PK], F32, tag="qrsb")
        delta = big.tile([128, NI, 4, H], F32, tag="delta")
        expb = big.tile([128, NI, H], F32, tag="expb")
        if honest_gate:
            gateT = big.tile([128, ND, L], BF16, tag="gateT")
        if use_mask:
            mb_t = big.tile([128, L], F32, tag="mbt")
            nc.sync.dma_start(out=mb_t, in_=_ap_bcast_rows(din["mbias"], 128))

        # ---------------- Stage A: LayerNorm (plain standardization) + transpose
        for src_ap, dstT in ((xs, xsT), (xt, xtT)):
            for it in range(NI):
                x_t = lnx.tile([128, D], F32, tag="lnx")
                nc.sync.dma_start(out=x_t, in_=src_ap[it * 128:(it + 1) * 128, :])
                st = stat.tile([128, 2, 6], F32, tag="st")
                for sg in range(2):
                    nc.vector.bn_stats(out=st[:, sg, :], in_=x_t[:, sg * 512:(sg + 1) * 512])
                mv = stat.tile([128, 2], F32, tag="mv")
                nc.vector.bn_aggr(out=mv, in_=st)
                rstd = stat.tile([128, 1], F32, tag="rstd")
                nc.scalar.activation(out=rstd, in_=mv[:, 1:2], func=AF.Sqrt, bias=eps_t)
                nc.vector.reciprocal(out=rstd, in_=rstd)
                nmr = stat.tile([128, 1], F32, tag="nmr")
                nc.vector.tensor_scalar(out=nmr, in0=mv[:, 0:1], scalar1=rstd,
                                        scalar2=-1.0, op0=ALU.mult, op1=ALU.mult)
                xhat = xh.tile([128, D], BF16, tag="xh")
                nc.scalar.activation(out=xhat, in_=x_t, func=AF.Identity, bias=nmr, scale=rstd)
                for blk in range(ND):
                    nc.sync.dma_start_transpose(
                        out=dstT[:, blk, it * 128:(it + 1) * 128],
                        in_=xhat[:, blk * 128:(blk + 1) * 128])

        # ---------------- Stage B: projections  yT[e,i] = sum_d W_T[d,e] x_T[d,i]
        # weights for one projection stay resident across its 8 e-tiles
        def project_resident(w_dram, rhsT, outT_t, bias_dram, act=AF.Identity,
                             headed=False):
            wts = [wp.tile([128, D], BF16, tag=f"w{db}", name=f"wt{db}") for db in range(ND)]
            for db in range(ND):
                nc.sync.dma_start(out=wts[db], in_=w_dram[db * 128:(db + 1) * 128, :])
            for et in range(ND):
                ps = ps_a.tile([128, L], F32, tag="ps_a")
                for db in range(ND):
                    nc.tensor.matmul(ps, lhsT=wts[db][:, et * 128:(et + 1) * 128],
                                     rhs=rhsT[:, db, :], start=(db == 0), stop=(db == ND - 1))
                b_t = tiny.tile([128, 1], F32, tag="bias")
                nc.sync.dma_start(out=b_t, in_=bias_dram[et * 128:(et + 1) * 128].rearrange("(p o) -> p o", o=1))
                if headed:
                    nc.scalar.activation(out=outT_t[:, 2 * et, :], in_=ps[0:64, :],
                                         func=act, bias=b_t[0:64, :])
                    nc.scalar.activation(out=outT_t[:, 2 * et + 1, :], in_=ps[64:128, :],
                                         func=act, bias=b_t[64:128, :])
                else:
                    nc.scalar.activation(out=outT_t[:, et, :], in_=ps, func=act, bias=b_t)

        project_resident(din["wqT"], xtT, qTh, din["qb"], headed=True)
        project_resident(din["wkT"], xsT, kTh, din["kb"], headed=True)
        project_resident(din["wv1T"], xtT, v1T, din["v1b"], act=AF.Relu)
        project_resident(din["wv2T"], v1T, vT, din["v2b"])
        if honest_gate:
            project_resident(din["wgT"], xsT, gateT, din["gb"], act=AF.Sigmoid)

        # v natural layout [j, d] for PV lhsT
        for jt in range(NI):
            for blk in range(ND):
                nc.sync.dma_start_transpose(
                    out=vnat[:, jt, blk * 128:(blk + 1) * 128],
                    in_=vT[:, blk, jt * 128:(jt + 1) * 128])

        # ---------------- Stage C: per i-tile masks + qr + deltas
        for it in range(NI):
            g_t = lnx.tile([128, L], F32, tag="gpm")
            nc.sync.dma_start(out=g_t, in_=gpm_f[it * 128:(it + 1) * 128, :])
            for p in range(4):
                nc.vector.tensor_scalar(out=masks[:, it, p, :], in0=g_t,
                                        scalar1=float(p), scalar2=None, op0=ALU.is_equal)
            qr_ps = ps_qr.tile([128, H * PK], F32, tag="qr")
            for h in range(H):
                nc.tensor.matmul(
                    qr_ps[:, h * PK:(h + 1) * PK],
                    lhsT=qTh[:, h, it * 128:(it + 1) * 128],
                    rhs=relkT2[0:64, :],
                    start=True, stop=True)
            nc.vector.tensor_copy(qrsb[:, it, :], qr_ps)
            # delta_p = qr_p - qr_4  (p<4, per head);  expb = qr_4 / 8
            qr_i = qrsb[:, it, :].rearrange("p (h k) -> p h k", k=PK)
            for p in range(4):
                nc.vector.tensor_tensor(out=delta[:, it, p, :], in0=qr_i[:, :, p],
                                        in1=qr_i[:, :, 4], op=ALU.subtract)
            nc.vector.tensor_scalar(out=expb[:, it, :], in0=qr_i[:, :, 4],
                                    scalar1=SCALE, scalar2=None, op0=ALU.mult)

        # ---------------- Stage D: attention, head pairs
        # Gather (relative-position bias) engine split per (h, i-tile):
        #  A: DVE fused scalar_tensor_tensor chain
        #  B: PE identity-matmul accumulation of DVE-prescaled masks into S psum
        #  C: GPSIMD tensor_tensor adds of DVE-prescaled masks (S pre-evicted by ACT)
        ident = const.tile([128, 128], BF16, name="ident")
        from concourse.kernels.tile_matmul import make_identity
        make_identity(nc, ident)
        CLS = "AAABBBCC"

        for g in range(ND):
            pt_tiles = {}
            for hf in range(2):
                h = 2 * g + hf
                pt = ptp.tile([128, NI, L], BF16, tag="pt")
                pt_tiles[hf] = pt
                for it in range(NI):
                    cls = CLS[(h * NI + it) % len(CLS)]
                    s_ps = ps_s.tile([128, L], F32, tag="s")
                    nc.tensor.matmul(
                        s_ps,
                        lhsT=qTh[:, h, it * 128:(it + 1) * 128],
                        rhs=kTh[:, h, :],
                        start=True, stop=(cls != "B"))
                    if cls == "A":
                        acc = accp.tile([128, L], F32, tag="acc")
                        nc.vector.scalar_tensor_tensor(
                            out=acc, in0=masks[:, it, 0, :],
                            scalar=delta[:, it, 0, h:h + 1], in1=s_ps,
                            op0=ALU.mult, op1=ALU.add)
                        for p in range(1, 4):
                            nc.vector.scalar_tensor_tensor(
                                out=acc, in0=masks[:, it, p, :],
                                scalar=delta[:, it, p, h:h + 1], in1=acc,
                                op0=ALU.mult, op1=ALU.add)
                        exp_in = acc
                    elif cls == "B":
                        for p in range(4):
                            e_t = accp.tile([128, L], BF16, tag="ebf")
                            nc.vector.tensor_scalar(out=e_t, in0=masks[:, it, p, :],
                                                    scalar1=delta[:, it, p, h:h + 1],
                                                    scalar2=None, op0=ALU.mult)
                            nc.tensor.matmul(s_ps, lhsT=ident, rhs=e_t,
                                             start=False, stop=(p == 3))
                        exp_in = s_ps
                    else:  # C
                        sev = sevp.tile([128, L], F32, tag="sev")
                        nc.scalar.activation(out=sev, in_=s_ps, func=AF.Copy)
                        acc = accp.tile([128, L], F32, tag="acc")
                        for p in range(4):
                            e_t = accp.tile([128, L], F32, tag="ef32")
                            nc.vector.tensor_scalar(out=e_t, in0=masks[:, it, p, :],
                                                    scalar1=delta[:, it, p, h:h + 1],
                                                    scalar2=None, op0=ALU.mult)
                            nc.gpsimd.tensor_tensor(out=acc, in0=(sev if p == 0 else acc),
                                                    in1=e_t, op=ALU.add)
                        exp_in = acc
                    if use_mask:
                        macc = accp.tile([128, L], F32, tag="acc")
                        nc.vector.tensor_tensor(out=macc, in0=exp_in, in1=mb_t, op=ALU.add)
                        exp_in = macc
                    p_t = pp.tile([128, L], BF16, tag="p")
                    rs = tiny.tile([128, 1], F32, tag="rs")
                    nc.scalar.activation(out=p_t, in_=exp_in, func=AF.Exp,
                                         bias=expb[:, it, h:h + 1], scale=SCALE,
                                         accum_out=rs)
                    r_t = tiny.tile([128, 1], F32, tag="r")
                    nc.vector.reciprocal(out=r_t, in_=rs)
                    pn = pp.tile([128, L], BF16, tag="pn")
                    nc.vector.tensor_scalar(out=pn, in0=p_t, scalar1=r_t,
                                            scalar2=None, op0=ALU.mult)
                    for jb in range(NI):
                        nc.sync.dma_start_transpose(
                            out=pt[:, jb, it * 128:(it + 1) * 128],
                            in_=pn[:, jb * 128:(jb + 1) * 128])
            o_ps = ps_o.tile([128, L], F32, tag="o")
            for hf in range(2):
                h = 2 * g + hf
                for jb in range(NI):
                    nc.tensor.matmul(
                        o_ps[hf * 64:(hf + 1) * 64, :],
                        lhsT=vnat[:, jb, h * 64:(h + 1) * 64],
                        rhs=pt_tiles[hf][:, jb, :],
                        start=(jb == 0), stop=(jb == NI - 1),
                        tile_position=(0, hf * 64))
            if honest_gate:
                og = pp.tile([128, L], BF16, tag="og")
                nc.scalar.activation(out=og, in_=o_ps, func=AF.Copy)
                nc.vector.tensor_tensor(out=outT[:, g, :], in0=og, in1=gateT[:, g, :],
                                        op=ALU.mult)
            else:
                nc.scalar.activation(out=outT[:, g, :], in_=o_ps, func=AF.Copy)

        # ---------------- Stage E: output projection
        wts = [wp.tile([128, D], BF16, tag=f"w{db}", name=f"wt{db}") for db in range(ND)]
        for db in range(ND):
            nc.sync.dma_start(out=wts[db], in_=din["woT"][db * 128:(db + 1) * 128, :])
        for et in range(ND):
            ps = ps_a.tile([128, L], F32, tag="ps_a")
            for db in range(ND):
                nc.tensor.matmul(ps, lhsT=wts[db][:, et * 128:(et + 1) * 128],
                                 rhs=outT[:, db, :], start=(db == 0), stop=(db == ND - 1))
            b_t = tiny.tile([128, 1], F32, tag="bias")
            nc.sync.dma_start(out=b_t, in_=din["ob"][et * 128:(et + 1) * 128].rearrange("(p o) -> p o", o=1))
            y_t = ytp.tile([128, L], F32, tag="yt")
            nc.scalar.activation(out=y_t, in_=ps, func=AF.Identity, bias=b_t)
            nc.sync.dma_start(out=yT[et * 128:(et + 1) * 128, :], in_=y_t)


def _host_prep(src, tgt, gpm, src_mask, ln_g, ln_b, q_w, k_w, v_w1, v_b1,
               v_w2, v_b2, rel_k, gate_w, gate_b, out_w, out_b):
    bf = ml_dtypes.bfloat16
    g = ln_g.astype(np.float64)
    b = ln_b.astype(np.float64)
    honest_gate = bool(np.any(gate_w))
    use_mask = not bool(np.all(src_mask))

    def foldT(w):
        return np.ascontiguousarray((w.astype(np.float64) * g[None, :]).T).astype(bf)

    wqT = foldT(q_w); wkT = foldT(k_w); wv1T = foldT(v_w1)
    wv2T = np.ascontiguousarray(v_w2.T).astype(bf)
    qb = (q_w.astype(np.float64) @ b).astype(np.float32)
    kb = (k_w.astype(np.float64) @ b).astype(np.float32)
    v1b = (v_b1.astype(np.float64) + v_w1.astype(np.float64) @ b).astype(np.float32)
    if honest_gate:
        gate0 = np.ones((D,), np.float64)
    else:
        gate0 = 1.0 / (1.0 + np.exp(-gate_b.astype(np.float64)))
    woT = np.ascontiguousarray((out_w.astype(np.float64) * gate0[None, :]).T).astype(bf)
    relkT2 = np.ascontiguousarray(np.concatenate([rel_k.T, rel_k.T], axis=0)).astype(bf)

    shared = dict(
        wqT=wqT, wkT=wkT, wv1T=wv1T, wv2T=wv2T, woT=woT, relkT2=relkT2,
        qb=qb, kb=kb, v1b=v1b, v2b=v_b2.astype(np.float32),
        ob=out_b.astype(np.float32),
    )
    if honest_gate:
        shared["wgT"] = foldT(gate_w)
        shared["gb"] = (gate_b.astype(np.float64) + gate_w.astype(np.float64) @ b).astype(np.float32)

    in_maps = []
    for c in range(B):
        m = dict(shared)
        m["xs"] = np.ascontiguousarray(src[c]).astype(np.float32)
        m["xt"] = np.ascontiguousarray(tgt[c]).astype(np.float32)
        m["gpm_f"] = gpm[c].astype(np.float32)
        if use_mask:
            m["mbias"] = np.where(src_mask[c], 0.0, -9e9).astype(np.float32)
        in_maps.append(m)
    return in_maps, honest_gate, use_mask


def get_program(honest_gate, use_mask, reps=1):
    key = (honest_gate, use_mask, reps)
    if key not in _PROG_CACHE:
        _PROG_CACHE[key] = build_program(honest_gate, use_mask, reps)
    return _PROG_CACHE[key]


def kernel(**inputs) -> np.ndarray:
    in_maps, honest_gate, use_mask = _host_prep(**inputs)
    nc = get_program(honest_gate, use_mask)
    res = run_bass_kernel_spmd(nc, in_maps, list(range(B)))
    out = np.stack([np.ascontiguousarray(res.results[c]["yT"].T) for c in range(B)], axis=0)
    return out
